# revision 5
# baseline (speedup 1.0000x reference)
"""CTMCell distributed kernel for 8 Trainium2 NeuronCores.

Sharding: data-parallel over batch (B=256 -> 32 per core); all params
(backbone/synapse dense weights, per-neuron SuperLinear weights, decay
params) replicated on every core. No collectives needed: each core
computes its batch shard end-to-end; outputs are concatenated on host.

Self-contained: shapes hardcoded, no sibling imports.
"""

import numpy as np
import jax
import jax.numpy as jnp
from functools import partial

B, OBS, DIN, D, M, HID, NSY, ITERS = 256, 256, 512, 1024, 32, 16, 64, 4
SYNCH = NSY * (NSY + 1) // 2  # 2080
NCORES = 8
BL = B // NCORES  # 32

_PARAM_NAMES = (
    'start_trace', 'start_activated_trace',
    'bb_w1', 'bb_b1', 'bb_ln1_s', 'bb_ln1_b',
    'bb_w2', 'bb_b2', 'bb_ln2_s', 'bb_ln2_b',
    'syn_w1', 'syn_b1', 'syn_ln1_s', 'syn_ln1_b',
    'syn_w2', 'syn_b2', 'syn_ln2_s', 'syn_ln2_b',
    'nlm1_w', 'nlm1_b', 'nlm1_T', 'nlm2_w', 'nlm2_b', 'nlm2_T',
)


def _decay_prep(decay_params):
    """Host-side: decay (M, SYNCH) scattered to dense (M, NSY, NSY) triu."""
    idx = np.arange(M - 1, -1, -1, dtype=np.float32)
    decay = np.exp(-idx[:, None] * np.clip(decay_params, 0.0, 4.0)[None, :])
    denom = np.sqrt(decay.sum(axis=0))  # (SYNCH,)
    ti, tj = np.triu_indices(NSY)
    decay_full = np.zeros((M, NSY, NSY), np.float32)
    decay_full[:, ti, tj] = decay
    return decay_full, (1.0 / denom).astype(np.float32)


def _glu(x):
    a, b = jnp.split(x, 2, axis=-1)
    return a * jax.nn.sigmoid(b)


def _ln(x, s, b):
    mu = jnp.mean(x, axis=-1, keepdims=True)
    var = jnp.var(x, axis=-1, keepdims=True)
    return (x - mu) / jnp.sqrt(var + 1e-6) * s + b


def _cell(obs, dones, st, at, params):
    (start_trace, start_activated_trace,
     bb_w1, bb_b1, bb_ln1_s, bb_ln1_b,
     bb_w2, bb_b2, bb_ln2_s, bb_ln2_b,
     syn_w1, syn_b1, syn_ln1_s, syn_ln1_b,
     syn_w2, syn_b2, syn_ln2_s, syn_ln2_b,
     nlm1_w, nlm1_b, nlm1_T, nlm2_w, nlm2_b, nlm2_T,
     decay_full, inv_denom) = params

    reset = dones[:, None, None]
    st = jnp.where(reset, start_trace[None], st)
    at = jnp.where(reset, start_activated_trace[None], at)

    f = _ln(_glu(obs @ bb_w1 + bb_b1), bb_ln1_s, bb_ln1_b)
    f = _ln(_glu(f @ bb_w2 + bb_b2), bb_ln2_s, bb_ln2_b)

    for _ in range(ITERS):
        last = at[:, :, -1]
        pre = jnp.concatenate([f, last], axis=-1)
        h = _ln(_glu(pre @ syn_w1 + syn_b1), syn_ln1_s, syn_ln1_b)
        h = _ln(_glu(h @ syn_w2 + syn_b2), syn_ln2_s, syn_ln2_b)
        st = jnp.concatenate([st[:, :, 1:], h[:, :, None]], axis=-1)
        x = jnp.einsum('BDM,MHD->BDH', st, nlm1_w) + nlm1_b
        x = _glu(x / nlm1_T)
        x = jnp.einsum('BDM,MHD->BDH', x, nlm2_w) + nlm2_b
        x = _glu(x / nlm2_T)[..., 0]
        at = jnp.concatenate([at[:, :, 1:], x[:, :, None]], axis=-1)

    # synch is overwritten every loop iteration in the reference; only the
    # value computed from the final `at` is returned, so compute it once.
    # Gather-free formulation: full outer product weighted by decay_full
    # (host-scattered to (M, NSY, NSY)), then static-slice triu packing.
    S = jnp.transpose(at[:, -NSY:, :], (0, 2, 1))  # (BL, M, NSY)
    full = jnp.einsum('bmi,bmj,mij->bij', S, S, decay_full)  # (BL, NSY, NSY)
    synch = jnp.concatenate([full[:, i, i:] for i in range(NSY)], axis=1)
    synch = synch * inv_denom[None, :]
    return st, at, synch


@partial(jax.pmap, axis_name='i', in_axes=(0, 0, 0, 0, None))
def _run(obs, dones, st, at, params):
    return _cell(obs, dones, st, at, params)


def kernel(**inputs):
    obs = np.ascontiguousarray(np.asarray(inputs['obs'], np.float32)).reshape(NCORES, BL, OBS)
    dones = np.ascontiguousarray(np.asarray(inputs['dones'], bool)).reshape(NCORES, BL)
    st = np.ascontiguousarray(np.asarray(inputs['state_trace'], np.float32)).reshape(NCORES, BL, D, M)
    at = np.ascontiguousarray(np.asarray(inputs['activated_state_trace'], np.float32)).reshape(NCORES, BL, D, M)
    decay_full, inv_denom = _decay_prep(np.asarray(inputs['decay_params'], np.float32))
    params = tuple(np.asarray(inputs[k], np.float32) for k in _PARAM_NAMES) + (decay_full, inv_denom)

    st_o, at_o, synch_o = _run(obs, dones, st, at, params)
    st_o = np.asarray(st_o).reshape(B, D, M)
    at_o = np.asarray(at_o).reshape(B, D, M)
    synch_o = np.asarray(synch_o).reshape(B, SYNCH)
    return st_o, at_o, synch_o


# revision 6
# speedup vs baseline: 34.5450x; 34.5450x over previous
"""CTMCell distributed kernel for 8 Trainium2 NeuronCores.

Sharding: data-parallel over batch (B=256 -> 32 per core); all params
(backbone/synapse dense weights, per-neuron SuperLinear weights, decay
params) replicated on every core. No collectives needed: each core
computes its batch shard end-to-end; outputs are concatenated on host.

Self-contained: shapes hardcoded, no sibling imports.
"""

import numpy as np
import jax
import jax.numpy as jnp
from functools import partial

B, OBS, DIN, D, M, HID, NSY, ITERS = 256, 256, 512, 1024, 32, 16, 64, 4
SYNCH = NSY * (NSY + 1) // 2  # 2080
NCORES = 8
BL = B // NCORES  # 32

_PARAM_NAMES = (
    'start_trace', 'start_activated_trace',
    'bb_w1', 'bb_b1', 'bb_ln1_s', 'bb_ln1_b',
    'bb_w2', 'bb_b2', 'bb_ln2_s', 'bb_ln2_b',
    'syn_w1', 'syn_b1', 'syn_ln1_s', 'syn_ln1_b',
    'syn_w2', 'syn_b2', 'syn_ln2_s', 'syn_ln2_b',
    'nlm1_w', 'nlm1_b', 'nlm1_T', 'nlm2_w', 'nlm2_b', 'nlm2_T',
)


def _decay_prep(decay_params):
    """Host-side: decay (M, SYNCH) scattered to dense (M, NSY, NSY) triu."""
    idx = np.arange(M - 1, -1, -1, dtype=np.float32)
    decay = np.exp(-idx[:, None] * np.clip(decay_params, 0.0, 4.0)[None, :])
    denom = np.sqrt(decay.sum(axis=0))  # (SYNCH,)
    ti, tj = np.triu_indices(NSY)
    decay_full = np.zeros((M, NSY, NSY), np.float32)
    decay_full[:, ti, tj] = decay
    return decay_full, (1.0 / denom).astype(np.float32)


def _glu(x):
    a, b = jnp.split(x, 2, axis=-1)
    return a * jax.nn.sigmoid(b)


def _ln(x, s, b):
    mu = jnp.mean(x, axis=-1, keepdims=True)
    var = jnp.var(x, axis=-1, keepdims=True)
    return (x - mu) / jnp.sqrt(var + 1e-6) * s + b


def _cell(obs, dones, st, at, params):
    (start_trace, start_activated_trace,
     bb_w1, bb_b1, bb_ln1_s, bb_ln1_b,
     bb_w2, bb_b2, bb_ln2_s, bb_ln2_b,
     syn_w1, syn_b1, syn_ln1_s, syn_ln1_b,
     syn_w2, syn_b2, syn_ln2_s, syn_ln2_b,
     nlm1_w, nlm1_b, nlm1_T, nlm2_w, nlm2_b, nlm2_T,
     decay_full, inv_denom) = params

    reset = dones[:, None, None]
    st = jnp.where(reset, start_trace[None], st)
    at = jnp.where(reset, start_activated_trace[None], at)

    f = _ln(_glu(obs @ bb_w1 + bb_b1), bb_ln1_s, bb_ln1_b)
    f = _ln(_glu(f @ bb_w2 + bb_b2), bb_ln2_s, bb_ln2_b)

    for _ in range(ITERS):
        last = at[:, :, -1]
        pre = jnp.concatenate([f, last], axis=-1)
        h = _ln(_glu(pre @ syn_w1 + syn_b1), syn_ln1_s, syn_ln1_b)
        h = _ln(_glu(h @ syn_w2 + syn_b2), syn_ln2_s, syn_ln2_b)
        st = jnp.concatenate([st[:, :, 1:], h[:, :, None]], axis=-1)
        x = jnp.einsum('BDM,MHD->BDH', st, nlm1_w) + nlm1_b
        x = _glu(x / nlm1_T)
        x = jnp.einsum('BDM,MHD->BDH', x, nlm2_w) + nlm2_b
        x = _glu(x / nlm2_T)[..., 0]
        at = jnp.concatenate([at[:, :, 1:], x[:, :, None]], axis=-1)

    # synch is overwritten every loop iteration in the reference; only the
    # value computed from the final `at` is returned, so compute it once.
    # Gather-free formulation: full outer product weighted by decay_full
    # (host-scattered to (M, NSY, NSY)), then static-slice triu packing.
    S = jnp.transpose(at[:, -NSY:, :], (0, 2, 1))  # (BL, M, NSY)
    full = jnp.einsum('bmi,bmj,mij->bij', S, S, decay_full)  # (BL, NSY, NSY)
    synch = jnp.concatenate([full[:, i, i:] for i in range(NSY)], axis=1)
    synch = synch * inv_denom[None, :]
    return st, at, synch


@partial(jax.pmap, axis_name='i', in_axes=(0, 0, 0, 0, 0))
def _run(obs, dones, st, at, params):
    return _cell(obs, dones, st, at, params)


def kernel(**inputs):
    obs = np.ascontiguousarray(np.asarray(inputs['obs'], np.float32)).reshape(NCORES, BL, OBS)
    dones = np.ascontiguousarray(np.asarray(inputs['dones'], bool)).reshape(NCORES, BL)
    st = np.ascontiguousarray(np.asarray(inputs['state_trace'], np.float32)).reshape(NCORES, BL, D, M)
    at = np.ascontiguousarray(np.asarray(inputs['activated_state_trace'], np.float32)).reshape(NCORES, BL, D, M)
    decay_full, inv_denom = _decay_prep(np.asarray(inputs['decay_params'], np.float32))
    params = tuple(np.asarray(inputs[k], np.float32) for k in _PARAM_NAMES) + (decay_full, inv_denom)

    devs = jax.devices()[:NCORES]
    shard = lambda x: jax.device_put_sharded([np.asarray(x[i]) for i in range(NCORES)], devs)
    obs, dones, st, at = shard(obs), shard(dones), shard(st), shard(at)
    params = tuple(jax.device_put_replicated(p, devs) for p in params)

    st_o, at_o, synch_o = _run(obs, dones, st, at, params)
    st_o = np.asarray(st_o).reshape(B, D, M)
    at_o = np.asarray(at_o).reshape(B, D, M)
    synch_o = np.asarray(synch_o).reshape(B, SYNCH)
    return st_o, at_o, synch_o
